# revision 2
# baseline (speedup 1.0000x reference)
"""GalaxyTileDecoder on 8 Trainium2 NeuronCores — canvas-view edition.

The reference pipeline (linear decode -> zero-pad -> gate -> bilinear
grid_sample -> sum over M=2 sources) collapses algebraically into

    out[p, i, j] = sum_s sum_{al,b,f} z[p,s,al,b,f] * canvas9[f, w_s+al+i, b+j]

where canvas9 is the 9 basis images (8 decoder rows + bias) in a 57x57
zero canvas, w_s = m_s + 2 in {0..4} is the integer y-shift slot of
source s, and z folds the bilinear weights, the galaxy_bool gate and the
decoder coefficients.

Device-side this is a matmul with contraction K = 108 = (al,b,f) per
distinct window w.  Three structural wins over materializing the full
324 x 2704 W_exp:

1. The moving operand is a strided VIEW of one small canvas tensor
   wcan[108, 57, 57], where partition (al,b,f) holds canvas9[f] flat
   pre-shifted by al*57+b (host-side).  A pass for window w reads
   wcan[:, w+i0:w+i1, 0:52] -- W input traffic drops from ~3MB to 702KB
   per core and EVERY window is available at no extra cost.

2. galaxy_bool zeroes ~30% of sources: ptiles with no active source are
   skipped entirely (host writes zeros); ptiles with one active source
   (or two sharing the same integer shift) need a single 108-row pass.
   A randomized-greedy packer builds 128-ptile-per-core batches whose
   union of windows is minimal: ~18 passes / 9 batches total vs the
   dense plan's 21 / 10.

3. PSUM is evacuated (f32 -> bf16) on three engines (DVE/ACT/GpSimd) so
   the faster PE stream is never gated on evacuation.

Output is written bf16 (~0.2% rounding, gate is 2e-2) and upcast on the
host.  Data parallel over ptiles, no collectives.
"""

import math
import os

import numpy as np

P_TOTAL = 10000
M = 2
N_CORES = 8
F = 9                            # 8 decoder features + bias
A_LOC = 2                        # y-taps per source
B = 6                            # x-shift positions
ROWS = A_LOC * B * F             # 108 contraction rows per pass
OUT_HW = 52
COLS = OUT_HW * OUT_HW           # 2704
CANVAS = 57
CAN2 = CANVAS * CANVAS           # 3249
BATCH = 128
UNIT = BATCH * N_CORES           # ptiles consumed by one batch across cores
N_WIN = 5

# segment rows (i-ranges): 6 PSUM banks, each <=512 f32 and 52-aligned
SEGROWS = [(0, 9), (9, 18), (18, 27), (27, 36), (36, 44), (44, 52)]
SEGS = [(r0 * OUT_HW, r1 * OUT_HW) for r0, r1 in SEGROWS]

_DT_NAME = os.environ.get("BASS_GAL_DT", "bf16")
WARM_MM = int(os.environ.get("BASS_GAL_WARM", "42"))

_cache = {}


def _build_program(dt_name, struct, vrs):
    """struct: tuple of window-tuples, one per batch, e.g. ((1,), (0, 3), ...).
    vrs: per-batch valid output rows (<= BATCH); only these are written."""
    import concourse.bass as bass  # noqa: F401  (registers engines)
    import concourse.tile as tile
    from concourse import bacc, mybir

    dt_map = {
        "bf16": mybir.dt.bfloat16,
        "f32": mybir.dt.float32,
    }
    DT = dt_map[dt_name]

    n_batches = len(struct)
    n_pass = sum(len(ws) for ws in struct)
    # flat pass -> zt column offset
    pass_col = []
    off = 0
    for ws in struct:
        cols = []
        for _ in ws:
            cols.append(off)
            off += BATCH
        pass_col.append(cols)

    nc = bacc.Bacc(trn_type="TRN2")
    zt = nc.dram_tensor("zt", [ROWS, n_pass * BATCH], DT, kind="ExternalInput")
    wr = nc.dram_tensor("wr", [ROWS, CANVAS, CANVAS], DT,
                        kind="ExternalInput")
    out = nc.dram_tensor("out", [n_batches * BATCH, COLS], mybir.dt.bfloat16,
                         kind="ExternalOutput")

    z_split = pass_col[0][-1] + BATCH       # cols for batch 0 only
    # cols for batches 1..2 (loaded up front); the rest is deferred until
    # batch 0 is in flight so it can't compete with the critical-path input
    zA_end = pass_col[min(2, n_batches - 1)][-1] + BATCH

    with tile.TileContext(nc) as tc:
        with (
            tc.tile_pool(name="w", bufs=1) as wpool,
            tc.tile_pool(name="o", bufs=4) as opool,
            tc.tile_pool(name="ps", bufs=8, space="PSUM") as pspool,
        ):
            # PE warmup: dummy matmuls bridging until the input DMAs land
            # (sustained PE busy also trips the HAM clock-gate to 2.4 GHz).
            warm = wpool.tile([128, 128], mybir.dt.bfloat16, tag="warm")
            nc.vector.memset(warm[:], 0.0)
            wps = pspool.tile([128, 512], mybir.dt.float32, tag="ps",
                              name="warmps")
            for _ in range(WARM_MM):
                nc.tensor.matmul(wps[:, 0:128], warm[:, 0:128], warm[:, 0:128],
                                 start=True, stop=True)

            # inputs on the critical path: batch 0's z block (scalar ring),
            # the canvas (sync ring), and z for batches 1-3 (scalar).  The
            # bulk z (zB) is deferred -- its DMA is gated on a 1-element
            # memset that the DVE only reaches once batch 0 is computing,
            # so its traffic never competes with the canvas.
            wc_t = wpool.tile([ROWS, CANVAS, CANVAS], DT, tag="wc")
            z0 = wpool.tile([ROWS, z_split], DT, tag="z0")
            zA = None
            zB = None
            nc.scalar.dma_start(z0[:], zt[:, 0:z_split])
            # canvas in two row-chunks: batch 0's first three segments only
            # read rows <= w+27 <= 31, so they can start on chunk 1
            nc.sync.dma_start(wc_t[:, 0:32, :], wr[:, 0:32, :])
            nc.sync.dma_start(wc_t[:, 32:CANVAS, :], wr[:, 32:CANVAS, :])
            if zA_end > z_split:
                zA = wpool.tile([ROWS, zA_end - z_split], DT, tag="zA")
                nc.scalar.dma_start(zA[:], zt[:, z_split:zA_end])
            if n_pass * BATCH > zA_end:
                zB = wpool.tile([ROWS, n_pass * BATCH - zA_end], DT, tag="zB")

            def z_slice(col):
                if col < z_split:
                    return z0[:, col:col + BATCH]
                if col < zA_end:
                    return zA[:, col - z_split:col - z_split + BATCH]
                return zB[:, col - zA_end:col - zA_end + BATCH]

            # PSUM evacuation: only DVE and ACT can read PSUM; alternate
            def evac(si, dst, src):
                if si % 2 == 0:
                    nc.vector.tensor_copy(dst, src)
                else:
                    nc.scalar.copy(dst, src)

            for bi, ws in enumerate(struct):
                last = bi == len(struct) - 1
                vr = vrs[bi]
                osb = opool.tile([128, COLS], mybir.dt.bfloat16, tag="osb")
                for si, ((r0, r1), (c0, c1)) in enumerate(zip(SEGROWS, SEGS)):
                    sw = c1 - c0
                    ps = pspool.tile([128, 512], mybir.dt.float32, tag="ps")
                    for ji, w in enumerate(ws):
                        zsl = z_slice(pass_col[bi][ji])
                        nc.tensor.matmul(
                            ps[0:128, 0:sw],
                            zsl,
                            wc_t[:, w + r0:w + r1, 0:OUT_HW],
                            start=(ji == 0),
                            stop=(ji == len(ws) - 1),
                        )
                    evac(si, osb[:, c0:c1], ps[0:128, 0:sw])
                    if bi == 0 and si == 0 and zB is not None:
                        # release the deferred bulk-z load: the memset (DVE,
                        # program order after batch 0's first evacuation)
                        # write-write gates the DMA trigger
                        nc.vector.memset(zB[0:1, 0:1], 0.0)
                        nc.sync.dma_start(zB[:], zt[:, zA_end:])
                    if last:
                        # tail: ship each segment as soon as it lands, the
                        # triggers alternating across both rings
                        eng = nc.sync if si % 2 == 0 else nc.scalar
                        eng.dma_start(out[bi * BATCH:bi * BATCH + vr, c0:c1],
                                      osb[0:vr, c0:c1])
                if not last:
                    # outputs ride the sync HWDGE ring: its sequencer runs no
                    # compute, so the trigger can't stall behind a next-batch
                    # evacuation the way it would on the scalar ring
                    nc.sync.dma_start(out[bi * BATCH:bi * BATCH + vr, :],
                                      osb[0:vr, :])
    nc.compile()
    return nc


def _get_program(dt_name, struct, vrs):
    key = (dt_name, struct, vrs)
    if key not in _cache:
        _cache[key] = _build_program(dt_name, struct, vrs)
    return _cache[key]


def _max_flow_assign(supplies, unit_types):
    """supplies: {wset: count}; unit_types: list of window-frozensets
    (capacity UNIT each).  Returns {(bucket, unit_idx): flow} if every
    supply can be placed, else None.  Tiny Ford-Fulkerson."""
    bks = list(supplies)
    nb, nu = len(bks), len(unit_types)
    # adjacency: bucket -> allowed units
    allowed = [[u for u in range(nu) if set(bks[b]) <= unit_types[u]]
               for b in range(nb)]
    if any(not a for a in allowed):
        return None
    flow = {}
    cap_u = [UNIT] * nu
    rem_b = [supplies[k] for k in bks]
    # greedy saturate, then augment via BFS on residual graph
    for b in range(nb):
        for u in allowed[b]:
            if rem_b[b] == 0:
                break
            f = min(rem_b[b], cap_u[u])
            if f:
                flow[(b, u)] = flow.get((b, u), 0) + f
                rem_b[b] -= f
                cap_u[u] -= f
    for b in range(nb):
        while rem_b[b] > 0:
            # BFS for augmenting path: b -> u1 -(take back from b2)-> u2...
            parent = {}
            frontier = [("b", b)]
            seen_u, seen_b = set(), {b}
            aug = None
            while frontier and aug is None:
                nxt = []
                for kind, x in frontier:
                    if kind == "b":
                        for u in allowed[x]:
                            if u in seen_u:
                                continue
                            seen_u.add(u)
                            parent[("u", u)] = ("b", x)
                            if cap_u[u] > 0:
                                aug = ("u", u)
                                break
                            nxt.append(("u", u))
                    else:
                        for b2 in range(nb):
                            if b2 in seen_b or flow.get((b2, x), 0) == 0:
                                continue
                            seen_b.add(b2)
                            parent[("b", b2)] = ("u", x)
                            nxt.append(("b", b2))
                    if aug:
                        break
                frontier = nxt
            if aug is None:
                return None
            # trace back, bottleneck
            path = [aug]
            while path[-1] != ("b", b):
                path.append(parent[path[-1]])
            path.reverse()  # b ... -> aug u
            bn = rem_b[b]
            for i in range(0, len(path) - 1, 2):
                _, bb = path[i]
                _, uu = path[i + 1]
                if i + 2 < len(path):
                    bn = min(bn, flow.get((path[i + 2][1], uu), 0))
            bn = min(bn, cap_u[path[-1][1]])
            assert bn > 0
            for i in range(0, len(path) - 1, 2):
                _, bb = path[i]
                _, uu = path[i + 1]
                flow[(bb, uu)] = flow.get((bb, uu), 0) + bn
                if i + 2 < len(path):
                    b2 = path[i + 2][1]
                    flow[(b2, uu)] -= bn
            rem_b[b] -= bn
            cap_u[path[-1][1]] -= bn
    return {(bks[b], u): f for (b, u), f in flow.items() if f > 0}


def _plan_units(wsets):
    """wsets: (P,) list of per-ptile sorted window tuples (len 0..2).

    Returns list of (windows_tuple, ids_array) units, each <= UNIT ptiles,
    total units = ceil(n_active/UNIT); unit windows cover every member's
    window set.  Full single-window units first, then an exact search over
    unit window-set multisets (cover + max-flow feasibility), minimizing
    total passes."""
    from itertools import combinations, combinations_with_replacement

    P = len(wsets)
    buckets = {}
    for p, ws in enumerate(wsets):
        if ws:
            buckets.setdefault(ws, []).append(p)
    buckets = {k: np.asarray(v) for k, v in buckets.items()}
    n_active = sum(len(v) for v in buckets.values())
    if n_active == 0:
        return []
    n_units = math.ceil(n_active / UNIT)

    rem = {k: len(v) for k, v in buckets.items()}
    units = []   # [windows_tuple, {bucket: count}]

    # full single-window units (always optimal: 1 pass each)
    for w in range(N_WIN):
        k = (w,)
        while rem.get(k, 0) >= UNIT:
            units.append([(w,), {k: UNIT}])
            rem[k] -= UNIT
    # full pair units
    for k in sorted(rem, key=lambda k: -rem[k]):
        if len(k) == 2:
            while rem[k] >= UNIT:
                units.append([k, {k: UNIT}])
                rem[k] -= UNIT

    supplies = {k: v for k, v in rem.items() if v > 0}
    r = n_units - len(units)
    assign = None
    if supplies and 0 < r <= 7:
        types = ([frozenset(c) for c in combinations(range(N_WIN), 1)]
                 + [frozenset(c) for c in combinations(range(N_WIN), 2)]
                 + [frozenset(c) for c in combinations(range(N_WIN), 3)])
        tmasks = [sum(1 << w for w in t) for t in types]
        bmasks = {k: sum(1 << w for w in k) for k in supplies}
        cands = sorted(combinations_with_replacement(range(len(types)), r),
                       key=lambda c: sum(len(types[i]) for i in c))
        for cand in cands:
            cms = [tmasks[i] for i in cand]
            if any(all(bm & ~tm for tm in cms) for bm in bmasks.values()):
                continue
            ut = [types[i] for i in cand]
            assign = _max_flow_assign(supplies, ut)
            if assign is not None:
                unit_takes = [dict() for _ in ut]
                for (k, u), f in assign.items():
                    unit_takes[u][k] = f
                for t, take in zip(ut, unit_takes):
                    if take:
                        units.append([tuple(sorted(t)), take])
                break
    if supplies and assign is None:
        # fallback: one unit per leftover chunk, window set = union
        flat = []
        for k, v in supplies.items():
            flat.extend([k] * v)
        for i in range(0, len(flat), UNIT):
            chunk = flat[i:i + UNIT]
            wins = set()
            take = {}
            for k in chunk:
                wins |= set(k)
                take[k] = take.get(k, 0) + 1
            units.append([tuple(sorted(wins)), take])

    # materialize ids per unit
    cursor = {k: 0 for k in buckets}
    out = []
    for wins, take in units:
        ids = []
        for k, n in take.items():
            c = cursor[k]
            ids.append(buckets[k][c:c + n])
            cursor[k] = c + n
        out.append((wins, np.concatenate(ids) if ids else np.empty(0, np.int64)))
    out = [u for u in out if len(u[1])]
    # execution order: a 1-pass unit first (it needs the least input data),
    # then alternate 1-pass (evacuation-bound) with multi-pass (PE-bound)
    # units so the copy engines never fall far behind, and END on the
    # heaviest units so the final evacuation isn't backlogged
    lows = sorted((u for u in out if len(u[0]) == 1), key=lambda u: -len(u[1]))
    highs = sorted((u for u in out if len(u[0]) > 1), key=lambda u: len(u[0]))
    order = []
    while lows or highs:
        if lows:
            order.append(lows.pop(0))
        if highs:
            order.append(highs.pop(0))
    return order


def _host_prepare(locs, galaxy_params, galaxy_bool, W_dec, b_dec, np_dtype):
    """Per-source coefficients, per-ptile window sets, shifted canvas."""
    locs = np.asarray(locs, np.float32).reshape(-1, 2)
    params = np.asarray(galaxy_params, np.float32).reshape(-1, 8)
    gbool = np.asarray(galaxy_bool, np.float32).reshape(-1)
    W = np.asarray(W_dec, np.float32)
    b = np.asarray(b_dec, np.float32)
    N = locs.shape[0]
    P = N // M

    sy = 2.5 - 4.0 * locs[:, 0]
    sx = 2.5 - 4.0 * locs[:, 1]
    m = np.clip(np.floor(sy), -2, 2)
    k = np.clip(np.floor(sx), -2, 2)
    fy = (sy - m).astype(np.float32)
    fx = (sx - k).astype(np.float32)
    w_src = (m + 2).astype(np.int64)          # window 0..4
    k = k.astype(np.int64)
    act = gbool != 0.0

    # per-source 108-row coefficient block, rows (al, f, b) -- matches the
    # on-chip canvas replication partition order
    ar = np.arange(N)
    cy = np.stack([1.0 - fy, fy], axis=1)                    # (N, 2)
    cx = np.zeros((N, B), np.float32)
    cx[ar, k + 2] = 1.0 - fx
    cx[ar, k + 3] = fx
    z9 = np.concatenate([params, np.ones((N, 1), np.float32)], axis=1)
    z9 *= gbool[:, None]
    zblk = (cy[:, :, None, None] * z9[:, None, :, None] * cx[:, None, None, :])
    zblk = zblk.reshape(N, ROWS)

    # per-ptile per-window coefficient sums (same-window pairs merge)
    pt_idx = np.repeat(np.arange(P), M)
    z_pw = np.zeros((P + 1, N_WIN, ROWS), np.float32)        # row P = pad
    np.add.at(z_pw, (pt_idx[act], w_src[act]), zblk[act])

    # per-ptile window set
    wsets = []
    wp = w_src.reshape(P, M)
    ap = act.reshape(P, M)
    for p in range(P):
        ws = sorted(set(wp[p, ap[p]].tolist()))
        wsets.append(tuple(ws))

    # pre-shifted canvas: partition p = al*54 + f*6 + b holds canvas9[f]
    # flat, shifted left by al*57+b and zero-padded
    canvas9 = np.zeros((F, CANVAS, CANVAS), np.float32)
    canvas9[:8, 3:54, 3:54] = W.reshape(8, 51, 51)
    canvas9[8, 3:54, 3:54] = b.reshape(51, 51)
    cflat = canvas9.reshape(F, CAN2)
    wcan = np.zeros((A_LOC, F, B, CAN2), np.float32)
    for al in range(A_LOC):
        for bb in range(B):
            sh = al * CANVAS + bb
            wcan[al, :, bb, :CAN2 - sh] = cflat[:, sh:]
    wcan = wcan.reshape(ROWS, CANVAS, CANVAS).astype(np_dtype)
    return z_pw, wsets, wcan


def kernel(locs, galaxy_params, galaxy_bool, W_dec, b_dec, _trace=False):
    import ml_dtypes
    from concourse.bass_utils import run_bass_kernel_spmd

    np_dtype = {
        "bf16": ml_dtypes.bfloat16,
        "f32": np.float32,
    }[_DT_NAME]

    z_pw, wsets, wcan = _host_prepare(
        locs, galaxy_params, galaxy_bool, W_dec, b_dec, np_dtype)
    P = len(wsets)
    units = _plan_units(wsets)

    struct = tuple(wins for wins, _ in units)
    n_batches = len(struct)
    n_pass = sum(len(ws) for ws in struct)

    # batch_ids: (n_batches, N_CORES, BATCH), -1 pad; each unit's ptiles are
    # spread evenly across cores so only vr rows per core hold data -- the
    # output DMA then ships just those rows
    # NOTE: partial-partition DMAs serialize onto a single SDMA engine
    # (hardware partition->engine swizzle), so every output ships all 128
    # rows; pad rows land in the host-side dump slot
    batch_ids = np.full((n_batches, N_CORES, BATCH), -1, np.int64)
    for i, (_, ids) in enumerate(units):
        vr = math.ceil(len(ids) / N_CORES)
        padded = np.full(N_CORES * vr, -1, np.int64)
        padded[:len(ids)] = ids
        batch_ids[i, :, :vr] = padded.reshape(N_CORES, vr)
    vrs = tuple(BATCH for _ in units)

    # per-core zt: [108, n_pass*128], one 128-col block per (batch, window)
    safe_ids = np.where(batch_ids < 0, P, batch_ids)
    zt = np.empty((N_CORES, ROWS, n_pass * BATCH), np_dtype)
    for c in range(N_CORES):
        off = 0
        for bi, ws in enumerate(struct):
            rows = safe_ids[bi, c]
            for w in ws:
                zt[c, :, off:off + BATCH] = z_pw[rows, w].T
                off += BATCH

    nc = _get_program(_DT_NAME, struct, vrs)
    in_maps = [{"zt": zt[c], "wr": wcan} for c in range(N_CORES)]
    kwargs = {}
    if _trace:
        kwargs["trace"] = True
    res = run_bass_kernel_spmd(nc, in_maps, core_ids=list(range(N_CORES)), **kwargs)

    full = np.zeros((P + 1, COLS), np.float32)
    for c in range(N_CORES):
        co = np.asarray(res.results[c]["out"]).astype(np.float32)
        full[safe_ids[:, c, :].reshape(-1)] = co.reshape(n_batches * BATCH, COLS)
    full[P] = 0.0
    out = full[:P].reshape(P, 1, OUT_HW, OUT_HW)
    if _trace:
        kernel._last_result = res
    return out, out
